# revision 36
# baseline (speedup 1.0000x reference)
"""Trainium2 Bass kernel for the CWLNFace margin-softmax loss head.

Reference computation (B=512, EMB=512, C=70722):
    kernel_norm = kernel / ||kernel||_col            # l2-normalize columns
    cosine      = clip(emb @ kernel_norm, -1+eps, 1-eps)
    out         = S * cos(clip(acos(cosine) - onehot*M*ms, eps, pi-eps))
                  - S * onehot*(M + M*ms)
For every non-label entry the acos/cos round-trip is the identity (the
theta clip never engages because |cosine| <= 1-eps keeps theta inside
[0.0447, pi-0.0447]), so the dense part is just  S * clip(cosine).  The
margin corrections touch exactly B=512 entries (one per row) and are
applied on the host from the device-computed clipped cosine values.

Device strategy (8 NeuronCores, classnum sharded):
    - Shard C across 8 cores (pad 70722 -> 8*8960 = 71680 with ones).
    - Each core computes  outT[c, b] = S * clip(dot(emb_b, k_c)/||k_c||)
      with the output transposed ([C_sh, B]) so the per-column norm scale
      is a per-partition scalar multiply.
    - Column norms via ACT Square (bf16) + PE matmul against a ones
      vector; main matmul in float32r (full PE rate at N=512).
Host reassembles, transposes, and patches the 512 label entries.
"""

import math
import numpy as np

B = 512
EMB = 512
C = 70722
NCORES = 8
CSH = 8960          # per-core padded classnum shard
NT = CSH // 128     # 70 C-tiles of 128 columns
S = 64.0
EPS = 1e-3
MARGIN = 0.4
H = 0.333
CLIP_HI = S * (1.0 - EPS)

_CACHE = {}


def _build_nc(reps=1):
    from contextlib import ExitStack

    from concourse import bacc, mybir, tile

    f32 = mybir.dt.float32
    bf16 = mybir.dt.bfloat16
    AF = mybir.ActivationFunctionType
    OP = mybir.AluOpType

    nc = bacc.Bacc(
        "TRN2",
        target_bir_lowering=False,
        debug=False,
        enable_asserts=False,
    )

    # Inputs arrive as bf16 (host-converted): the matmul consumes bf16
    # anyway, so shipping bf16 halves the input DMA bytes and removes the
    # on-chip convert pass.
    embT = nc.dram_tensor("embT", [EMB, B], bf16, kind="ExternalInput").ap()
    # Host pre-tiles the shard so each C-tile is one contiguous 128 KiB
    # block: [tile, partition(EMB%128), chunk(EMB//128), col].
    # Macro-major layouts: each load/store is one fully contiguous block
    # with 2 KiB per-partition runs (vs 1 KiB when row-major).
    ksh = nc.dram_tensor(
        "ksh", [NT // 2, 128, 2, 4, 128], bf16, kind="ExternalInput"
    ).ap()
    # Output in bf16 (host upconverts): the values are bf16-matmul-limited
    # (~2.7e-3) anyway, and this halves the dominant DMA stream.
    out = nc.dram_tensor(
        "out", [NT // 2, 128, 2, B], bf16, kind="ExternalOutput"
    ).ap()

    with tile.TileContext(nc) as tc, ExitStack() as ctx:
        singles = ctx.enter_context(tc.tile_pool(name="singles", bufs=1))
        kpool = ctx.enter_context(tc.tile_pool(name="k", bufs=10))
        sqpool = ctx.enter_context(tc.tile_pool(name="sq", bufs=6))
        opool = ctx.enter_context(tc.tile_pool(name="o", bufs=8))
        scpool = ctx.enter_context(tc.tile_pool(name="sc", bufs=12))
        # pc tiles are 2 PSUM banks each now: 3*2 + 2*1 = 8 banks total
        pcpool = ctx.enter_context(tc.tile_pool(name="pc", bufs=3, space="PSUM"))
        pnpool = ctx.enter_context(tc.tile_pool(name="pn", bufs=2, space="PSUM"))

        # Embeddings^T resident in SBUF: [128, chunk, B], chunk = EMB/128.
        emb_sb = singles.tile([128, 4, B], bf16)
        nc.sync.dma_start(
            out=emb_sb[:], in_=embT.rearrange("(c p) b -> p c b", p=128)
        )
        ones_sb = singles.tile([128, 1], bf16)
        nc.vector.memset(ones_sb[:], 1.0)

        # Macro-tiles of 2 C-tiles: batches the small ACT/DVE ops (352-cycle
        # ACT fixed overhead, DVE instruction overhead) and halves the DMA
        # instruction count.
        NM = NT // 2
        for m in [m for _ in range(reps) for m in range(NM)]:
            # Input loads on the ACT HWDGE ring, output stores on the SP
            # ring: one sequencer issuing both would execute out[t]'s data
            # wait inline and block the issue of in[t+1] behind the whole
            # compute chain, capping the pipeline at ~2 tiles.
            kb_t = kpool.tile([128, 2, 4, 128], bf16)
            nc.scalar.dma_start(out=kb_t[:], in_=ksh[m])

            # Column sum-of-squares via PE: normsq[c] = sum_k sq[k, c].
            sq_t = sqpool.tile([128, 2, 4, 128], bf16)
            nc.vector.tensor_mul(sq_t[:], kb_t[:], kb_t[:])
            pn = pnpool.tile([128, 2], f32)
            for u in range(2):
                for c in range(4):
                    nc.tensor.matmul(
                        pn[:, u : u + 1],
                        lhsT=sq_t[:, u, c, :],
                        rhs=ones_sb[:],
                        start=(c == 0),
                        stop=(c == 3),
                    )
            # scale = S / sqrt(normsq)
            r_t = scpool.tile([128, 2], f32)
            nc.vector.reciprocal(r_t[:], pn[:])
            sc_t = scpool.tile([128, 2], f32)
            nc.scalar.activation(sc_t[:], r_t[:], AF.Sqrt, scale=S * S)

            # Main matmul: cosT_tile = ksh_tile^T @ embT  ([128 C, 512 B]),
            # ACT per-partition scale copy (PSUM->SBUF) per sub-tile.
            pc = pcpool.tile([128, 2, B], f32)
            o_t = opool.tile([128, 2, B], bf16)
            for u in range(2):
                for c in range(4):
                    nc.tensor.matmul(
                        pc[:, u, :],
                        lhsT=kb_t[:, u, c, :],
                        rhs=emb_sb[:, c, :],
                        start=(c == 0),
                        stop=(c == 3),
                    )
                nc.scalar.activation(
                    o_t[:, u, :], pc[:, u, :], AF.Copy, scale=sc_t[:, u : u + 1]
                )
            # clip both sub-tiles in one DVE pass
            nc.vector.tensor_scalar(
                o_t[:], o_t[:], CLIP_HI, -CLIP_HI, OP.min, OP.max
            )
            nc.sync.dma_start(out=out[m], in_=o_t[:])

    nc.compile()
    return nc


def _get_nc():
    if "nc" not in _CACHE:
        _CACHE["nc"] = _build_nc()
    return _CACHE["nc"]


def make_shards(kfull):
    """Split kernel [EMB, C] into per-core tile-major bf16 shards
    [NT, 128, 4, 128] (each C-tile contiguous)."""
    import ml_dtypes

    bf16 = np.dtype(ml_dtypes.bfloat16)
    shards = []
    for i in range(NCORES):
        lo, hi = i * CSH, (i + 1) * CSH
        if hi <= C:
            shard = kfull[:, lo:hi].astype(bf16)
        else:
            shard = np.ones((EMB, CSH), dtype=bf16)
            shard[:, : C - lo] = kfull[:, lo:C].astype(bf16)
        # rows = (chunk, p), cols = (macro, sub, w) -> [macro, p, sub, chunk, w]
        tiled = shard.reshape(4, 128, NT // 2, 2, 128).transpose(2, 1, 3, 0, 4)
        shards.append(np.ascontiguousarray(tiled))
    return shards


def run_device(embbedings, kernel, trace=False):
    """Run the sharded device kernel. Returns (outT [C,B] float32, results)."""
    from concourse.bass_utils import run_bass_kernel_spmd

    nc = _get_nc()

    import ml_dtypes

    embT = np.ascontiguousarray(
        np.asarray(embbedings, dtype=np.float32).T.astype(ml_dtypes.bfloat16)
    )
    kfull = np.asarray(kernel, dtype=np.float32)

    in_maps = [
        {"embT": embT, "ksh": shard} for shard in make_shards(kfull)
    ]

    res = run_bass_kernel_spmd(nc, in_maps, core_ids=list(range(NCORES)), trace=trace)
    # per-core out is [NM, 128, 2, B] macro-major -> row-major [CSH, B]
    parts = [
        np.asarray(r["out"]).transpose(0, 2, 1, 3).reshape(CSH, B)
        for r in res.results
    ]
    outT = np.concatenate(parts, axis=0)[:C].astype(np.float32)  # [C, B]
    return outT, res


def kernel(embbedings, norms, label, class_sample_num_, kernel):
    outT, _ = run_device(embbedings, kernel)

    # ---- host margin fix-up (touches exactly B entries) ----
    norms = np.asarray(norms, dtype=np.float32)
    csn = np.asarray(class_sample_num_, dtype=np.float32)
    lab = np.asarray(label).astype(np.int64)

    safe = np.clip(norms, 0.001, 100.0)
    safe = safe / (csn[:, None] + 0.001)
    safe = np.clip(safe, 0.001, 100.0).astype(np.float32)
    mean = safe.mean(dtype=np.float64)
    std = safe.std(ddof=1, dtype=np.float64)
    ms = np.clip((safe.astype(np.float64) - mean) / (std + EPS) * H, -1.0, 1.0)[:, 0]

    # Exact label-column cosines on the host (512 length-512 dots): the
    # device's bf16 values would be amplified ~22x by arccos near the clip
    # boundary, so recompute them at full precision.
    rows = np.arange(B)
    emb64 = np.asarray(embbedings, dtype=np.float64)
    cols = np.asarray(kernel, dtype=np.float64)[:, lab]  # [EMB, B]
    dots = np.einsum("be,eb->b", emb64, cols)
    c0 = np.clip(dots / np.linalg.norm(cols, axis=0), -1.0 + EPS, 1.0 - EPS)
    theta = np.arccos(c0) - MARGIN * ms
    theta = np.clip(theta, EPS, math.pi - EPS)
    val = (np.cos(theta) - (MARGIN + MARGIN * ms)) * S
    outT[lab, rows] = val.astype(np.float32)

    return np.ascontiguousarray(outT.T)
